# revision 3
# baseline (speedup 1.0000x reference)
"""Trainium2 Bass kernel for nn_GetNodeK (gnn_message_passing).

out[b,i,n,m,:] = node_embedding[b, nbr_idx[b, nbr_idx[b,i,n], m], :]

Sharding: data-parallel over B (8 batches -> 8 cores, one batch per core).

Let nbr_flat = nbr_idx[b].reshape(6144) (values < 256) and define the
one-hop table G[j] = concat_m emb[nbr[j,m]] (256 rows x 12 KB = 3.1 MB).
Then out[b, k=(i*24+n)] = G[nbr_flat[k]] -- the 2-hop gather factors into
two index-driven stages that both use the raw nbr values (no chained
index arithmetic anywhere).

v2 (default): stage 1 dma_gather emb->G in SBUF (permuted so scatter-token
j sits at partition j%128, half j//128, 12 KB contiguous); stage 2 is
T = max_j count(j) rounds of indirect_dma_start scatter SBUF->DRAM where
round r writes G[j] to the r-th output row that references j (OOB-skip
via bounds_check for exhausted tokens). HBM traffic: 75.5 MB write +
3.1 MB read per core (roofline-ish).

v1 (fallback): stage 1 gather -> G -> DRAM; stage 2 dma_gather 12 KB rows
from G_dram -> SBUF tiles -> sequential DMA out. Extra 75.5 MB read.
"""
import numpy as np

from concourse import bass, bacc, mybir
import concourse.tile as tile
from concourse.bass_utils import run_bass_kernel_spmd

B, At, Nbr, F = 8, 256, 24, 128
NI = At * Nbr        # 6144 indices per batch
ROW = Nbr * F        # 3072 f32 = 12 KB per stage-2 row
CH = 512             # v1 stage-2 chunk (indices per gather call)
NCHUNK = NI // CH    # 12
OOB = 8192           # idx sentinel > NI-1 -> skipped by bounds_check

VERSION = "v2"
_CACHED = {}


# ---------------------------------------------------------------- v1 ----
def _build_nc_v1():
    nc = bacc.Bacc("TRN2", target_bir_lowering=False, debug=False)
    emb = nc.dram_tensor("emb", [At, F], mybir.dt.float32, kind="ExternalInput")
    gidx = nc.dram_tensor("gidx", [128, NI // 16], mybir.dt.int16, kind="ExternalInput")
    g_dram = nc.dram_tensor("g_scratch", [NI, F], mybir.dt.float32)
    out = nc.dram_tensor("out", [NI, ROW], mybir.dt.float32, kind="ExternalOutput")

    with tile.TileContext(nc) as tc:
        with tc.tile_pool(name="pool0", bufs=1) as pool0, \
             tc.tile_pool(name="pool2", bufs=2) as pool2:
            idx_t = pool0.tile([128, NI // 16], mybir.dt.int16)
            nc.sync.dma_start(idx_t[:], gidx[:])

            g_t = pool0.tile([128, NI // 128, F], mybir.dt.float32)
            nc.gpsimd.dma_gather(g_t[:], emb[:], idx_t[:], NI, NI, F,
                                 single_packet=False)
            nc.sync.dma_start(
                g_dram[:].rearrange("(s p) e -> p s e", p=128), g_t[:]
            )

            g_view = g_dram[:].rearrange("(j k) e -> j (k e)", k=Nbr)  # [256, 3072]
            for c in range(NCHUNK):
                t2 = pool2.tile([128, CH // 128, ROW], mybir.dt.float32, tag="t2")
                nc.gpsimd.dma_gather(
                    t2[:], g_view,
                    idx_t[:, c * (CH // 16):(c + 1) * (CH // 16)],
                    CH, CH, ROW,
                )
                nc.sync.dma_start(
                    out[c * CH:(c + 1) * CH].rearrange("(s p) e -> p s e", p=128),
                    t2[:],
                )
    nc.compile()
    return nc


def _prep_v1(nbr16_b):
    flat = nbr16_b.reshape(-1)
    return {"gidx": np.tile(flat.reshape(NI // 16, 16).T, (8, 1))}


# ---------------------------------------------------------------- v2 ----
_T_PERM = None


def _v1_perm():
    """idx1[t] = nbr[(t//128//24)*128 + t%128, (t//128)%24] as flat index."""
    global _T_PERM
    if _T_PERM is None:
        t = np.arange(NI)
        s, p = t // 128, t % 128
        j, m = (s // Nbr) * 128 + p, s % Nbr
        _T_PERM = j * Nbr + m
    return _T_PERM


def _prep_v2(nbr16_b, T):
    flat = nbr16_b.reshape(-1)
    idx1 = flat[_v1_perm()]
    gidx = np.tile(idx1.reshape(NI // 16, 16).T, (8, 1))

    counts = np.bincount(flat, minlength=At)
    order = np.argsort(flat, kind="stable")
    tbl = np.full((At, T), OOB, dtype=np.int32)
    pos = 0
    for j in range(At):
        c = counts[j]
        tbl[j, :c] = order[pos:pos + c]
        pos += c
    sidx = np.empty((128, T, 2), dtype=np.int32)
    for q in range(2):
        sidx[:, :, q] = tbl[q * 128:(q + 1) * 128, :]
    return {"gidx": gidx, "sidx": sidx}


def _build_nc_v2(T):
    nc = bacc.Bacc("TRN2", target_bir_lowering=False, debug=False)
    emb = nc.dram_tensor("emb", [At, F], mybir.dt.float32, kind="ExternalInput")
    gidx = nc.dram_tensor("gidx", [128, NI // 16], mybir.dt.int16, kind="ExternalInput")
    sidx = nc.dram_tensor("sidx", [128, T, 2], mybir.dt.int32, kind="ExternalInput")
    out = nc.dram_tensor("out", [NI, ROW], mybir.dt.float32, kind="ExternalOutput")

    with tile.TileContext(nc) as tc:
        with tc.tile_pool(name="pool0", bufs=1) as pool0:
            idx_t = pool0.tile([128, NI // 16], mybir.dt.int16)
            nc.sync.dma_start(idx_t[:], gidx[:])
            sidx_t = pool0.tile([128, T, 2], mybir.dt.int32)
            nc.sync.dma_start(sidx_t[:], sidx[:])

            g_t = pool0.tile([128, NI // 128, F], mybir.dt.float32)
            nc.gpsimd.dma_gather(g_t[:], emb[:], idx_t[:], NI, NI, F,
                                 single_packet=False)

            g_scatter = g_t[:].rearrange("p (q m) e -> p q (m e)", q=2)
            for r in range(T):
                for q in range(2):
                    nc.gpsimd.indirect_dma_start(
                        out=out[:],
                        out_offset=bass.IndirectOffsetOnAxis(
                            ap=sidx_t[:, r, q:q + 1], axis=0),
                        in_=g_scatter[:, q, :],
                        in_offset=None,
                        bounds_check=NI - 1,
                        oob_is_err=False,
                    )
    nc.compile()
    return nc


# ------------------------------------------------------------- driver ----
def _run(nc, in_maps, **kwargs):
    return run_bass_kernel_spmd(nc, in_maps, core_ids=list(range(B)), **kwargs)


def kernel(node_embedding: np.ndarray, nbr_idx: np.ndarray, _collect=None) -> np.ndarray:
    node_embedding = np.ascontiguousarray(node_embedding, dtype=np.float32)
    nbr16 = nbr_idx.astype(np.int16)  # values in [0, 256)

    if VERSION == "v1":
        if "v1" not in _CACHED:
            _CACHED["v1"] = _build_nc_v1()
        nc = _CACHED["v1"]
        in_maps = [{"emb": node_embedding[b], **_prep_v1(nbr16[b])}
                   for b in range(B)]
    else:
        T = int(max(np.bincount(nbr16[b].reshape(-1), minlength=At).max()
                    for b in range(B)))
        key = ("v2", T)
        if key not in _CACHED:
            _CACHED[key] = _build_nc_v2(T)
        nc = _CACHED[key]
        in_maps = [{"emb": node_embedding[b], **_prep_v2(nbr16[b], T)}
                   for b in range(B)]

    res = _run(nc, in_maps)
    if _collect is not None:
        _collect.append(res)
    outs = [res.results[b]["out"].reshape(At, Nbr, Nbr, F) for b in range(B)]
    return np.stack(outs, axis=0)
